# revision 20
# baseline (speedup 1.0000x reference)
"""BiLSTM-CRF Trainium2 kernel (8-core SPMD, batch-sharded).

Per core: 4 sequences, full pipeline on device:
  embedding gather (indirect DMA) -> PE transposes -> input-gate GEMMs ->
  512-step bidirectional LSTM recurrence -> emission GEMM ->
  chunked max-plus Viterbi scan -> batched pointer extraction ->
  chunked backtrace via composed pointer maps -> int32 tag path.

Key performance structure vs the naive formulation:
  * The input GEMMs use float32r (single PE pass, full-rate with a 512
    moving dim); the LSTM recurrence uses float16 operands (same 10-bit
    mantissa as tf32) in 32-col quadrant matmuls whose weight loads
    pipeline across PE sub-array strips, halving the fp32 2-pass cost.
  * The Viterbi scan is chunked: 32 chunks x 16 steps. Within-chunk
    max-plus matrix prefix products are computed with all 128 chunks
    (chunk, seq) batched on partitions (15 serial steps instead of 512),
    followed by a tiny serial cross-chunk vector scan and a batched
    prefix application.
  * The backtrace composes pointer maps the same way: within-chunk map
    composition (one-hot gather algebra), serial chunk-boundary
    evaluation, then a batched apply.

Math notes:
  sigmoid(x) = 0.5*tanh(0.5x)+0.5 so every gate uses one Tanh activation;
  the 0.5 factors are pre-folded into the weights. Cell/hidden state are
  carried doubled (C=2c, H=2h); the 0.5 for H is folded into W_hh/W_out.
"""

import numpy as np

import concourse.bass as bass
import concourse.tile as tile
from concourse import bacc, mybir
from concourse.bass_utils import run_bass_kernel_spmd

FP = mybir.dt.float32
FPR = mybir.dt.float32r
F16 = mybir.dt.float16
I32 = mybir.dt.int32
AX = mybir.AxisListType
OP = mybir.AluOpType
AF = mybir.ActivationFunctionType

VOCAB = 100000
E = 256
Hh = 128
K = 12
START = 9
STOP = 10
NEG = -10000.0
B = 32
NCORES = 8
BL = B // NCORES  # 4 sequences per core
LCH = 16          # Viterbi chunk length
KK = K * K


def build_program(T=512):
    nc = bacc.Bacc("TRN2", target_bir_lowering=False, debug=False)
    NTOK = T * BL              # tokens per core
    NTILE = NTOK // 128        # gather tiles (16 at T=512)
    NCHUNK = NTOK // 512       # 512-col GEMM chunks (4)
    CH = T // LCH              # Viterbi chunks (32); CH*BL == 128

    def din(name, shape, dtype=FP):
        return nc.dram_tensor(name, list(shape), dtype, kind="ExternalInput").ap()

    idx_in = din("idx_in", [128, NTILE], I32)          # [p,k] token ids, time-major
    embed = din("embed", [VOCAB, E])
    w_ihT = din("w_ihT", [2, E, 4 * Hh], FPR)               # pre-scaled, gate order i,f,o,g
    w_hhT = din("w_hhT", [2, Hh, 4 * Hh], F16)
    b_in = din("b_in", [128, 8])                       # col d*4+g: per-partition bias
    h_init = din("h_init", [2, 128, BL], F16)               # 2*h0, feature-major
    c_init = din("c_init", [2, 128, BL])               # 2*c0
    w_outT = din("w_outT", [2, Hh, K], F16)                 # 0.5*W_out halves, transposed
    bout_rep = din("bout_rep", [128, K])
    ident = din("ident", [128, 128])
    identr = din("identr", [128, 128], F16)
    trans128 = din("trans128", [128, KK])              # trans[j,m] flat, replicated
    wvec128 = din("wvec128", [128, K])                 # 11-k, replicated
    tstop = din("tstop", [BL, K])                      # trans[STOP,:] replicated
    scores0 = din("scores0", [BL, K])
    g0_in = din("g0_in", [128, KK])                    # max-plus identity, replicated
    tr9_in = din("tr9_in", [BL, K])                    # trans[:,START] replicated
    iota_in = din("iota_in", [128, K])                 # 0..11 replicated

    path_out = nc.dram_tensor("path_out", [BL, T], I32, kind="ExternalOutput").ap()

    # DRAM scratch for partition-permute bounces
    d_mid = nc.dram_tensor("d_mid", [128, LCH * K], FP).ap()
    d_g15 = nc.dram_tensor("d_g15", [128, KK], FP).ap()
    d_u = nc.dram_tensor("d_u", [BL, CH * K], FP).ap()
    d_v = nc.dram_tensor("d_v", [BL, CH * K], FP).ap()
    d_c0 = nc.dram_tensor("d_c0", [128, K], FP).ap()
    d_x = nc.dram_tensor("d_x", [BL, CH], FP).ap()
    d_tags = nc.dram_tensor("d_tags", [CH, BL, LCH], I32).ap()

    with tile.TileContext(nc) as tc:
        with tc.tile_pool(name="const", bufs=1) as cpool, \
             tc.tile_pool(name="big", bufs=1) as bpool:

            # ---- load constants ----
            def cload(ap_in, shape, dtype=FP):
                t = cpool.tile(list(shape), dtype, name=f"c_{np.random.randint(1<<30)}")
                nc.sync.dma_start(t[:], ap_in)
                return t

            idx_sb = cload(idx_in, [128, NTILE], I32)
            wih_sb = [[cload(w_ihT[d, e * 128:(e + 1) * 128, :], [128, 4 * Hh], FPR)
                       for e in range(2)] for d in range(2)]
            whh_sb = [cload(w_hhT[d], [Hh, 4 * Hh], F16) for d in range(2)]
            b_sb = cload(b_in, [128, 8])
            hi_sb = [cload(h_init[d], [128, BL], F16) for d in range(2)]
            ci_sb = [cload(c_init[d], [128, BL]) for d in range(2)]
            wout_sb = [cload(w_outT[d], [Hh, K], F16) for d in range(2)]
            bout_sb = cload(bout_rep, [128, K])
            id_sb = cload(ident, [128, 128])
            idr_sb = cload(identr, [128, 128], F16)
            tr_sb = cload(trans128, [128, KK])
            wv_sb = cload(wvec128, [128, K])
            ts_sb = cload(tstop, [BL, K])
            s0_sb = cload(scores0, [BL, K])
            g0_sb = cload(g0_in, [128, KK])
            tr9_sb = cload(tr9_in, [BL, K])
            io_sb = cload(iota_in, [128, K])

            # big persistent arrays
            xg_sb = [bpool.tile([128, T * 16], F16, tag=f"xg{d}", name=f"xg{d}") for d in range(2)]
            hs_sb = [bpool.tile([128, T * BL], F16, tag=f"hs{d}", name=f"hs{d}") for d in range(2)]

            # ---- phase 1: embedding gather + transpose to [E, tok] ----
            with tc.tile_pool(name="gat", bufs=3) as gpool, \
                 tc.tile_pool(name="ps1", bufs=4, space="PSUM") as ps1, \
                 tc.tile_pool(name="xe", bufs=1) as xepool:
                xe_sb = [xepool.tile([128, NTOK], FPR, tag=f"xe{e}", name=f"xe{e}") for e in range(2)]
                for k in range(NTILE):
                    gt = gpool.tile([128, E], FP)
                    nc.gpsimd.indirect_dma_start(
                        out=gt[:],
                        out_offset=None,
                        in_=embed[:],
                        in_offset=bass.IndirectOffsetOnAxis(
                            ap=idx_sb[:, k:k + 1], axis=0),
                    )
                    for e in range(2):
                        pt = ps1.tile([128, 128], FP, space="PSUM")
                        nc.tensor.transpose(
                            out=pt[:], in_=gt[:, e * 128:(e + 1) * 128],
                            identity=id_sb[:])
                        nc.vector.tensor_copy(
                            xe_sb[e][:, k * 128:(k + 1) * 128], pt[:])

                # ---- phase 2: xg = W_ih_eff @ xe + b, interleaved [t,(g,b)] ----
                with tc.tile_pool(name="ps2", bufs=3, space="PSUM") as ps2:
                    for d in range(2):
                        xgv = xg_sb[d][:].rearrange("p (t x) -> p t x", x=16)
                        for g in range(4):
                            for c in range(NCHUNK):
                                pt = ps2.tile([128, 512], FP, space="PSUM")
                                for e in range(2):
                                    nc.tensor.matmul(
                                        pt[:],
                                        lhsT=wih_sb[d][e][:, g * 128:(g + 1) * 128],
                                        rhs=xe_sb[e][:, c * 512:(c + 1) * 512],
                                        start=(e == 0), stop=(e == 1),
                                    )
                                nc.vector.tensor_scalar(
                                    out=xgv[:, c * 128:(c + 1) * 128,
                                            g * 4:(g + 1) * 4],
                                    in0=pt[:].rearrange("p (t b) -> p t b", b=BL),
                                    scalar1=b_sb[:, d * 4 + g:d * 4 + g + 1],
                                    scalar2=None,
                                    op0=OP.add,
                                )

            # ---- phase 3: LSTM recurrence, two staggered chains ----
            # gate cols per step: i=0:4, f=4:8, o=8:12, g=12:16
            # Emission order is chosen so each engine's FIFO matches the
            # arrival order of a half-step-staggered steady state:
            # PE [inj0 whh0 inj1 whh1], ACT [tanh0 tanh1 tc0 tc1],
            # DVE [a0 b0 c0 a1 h0 b1 c1 h1].
            with tc.tile_pool(name="ps3", bufs=4, space="PSUM") as ps3, \
                 tc.tile_pool(name="th", bufs=4) as thpool, \
                 tc.tile_pool(name="cell", bufs=4) as cellpool, \
                 tc.tile_pool(name="cst", bufs=2) as cstpool:
                c_cur = [ci_sb[0], ci_sb[1]]
                for step in range(T):
                    tt = [step, T - 1 - step]
                    prev = [hi_sb[d][:] if step == 0 else
                            hs_sb[d][:, (tt[d] - 1 + 2 * d) * BL:
                                      (tt[d] + 2 * d) * BL]
                            for d in range(2)]
                    pt = []
                    for d in range(2):
                        p = ps3.tile([128, 20], FP, space="PSUM",
                                     tag=f"g{d}", name=f"g{d}_{step}")
                        pt.append(p)
                        for q in range(4):
                            nc.tensor.matmul(
                                p[32 * q:32 * (q + 1), 0:16],
                                lhsT=idr_sb[:, 32 * q:32 * (q + 1)],
                                rhs=xg_sb[d][:, tt[d] * 16:(tt[d] + 1) * 16],
                                start=True, stop=False,
                                tile_position=(0, 32 * q),
                                skip_group_check=True)
                    for d in range(2):
                        for g in range(4):
                            for q in range(4):
                                nc.tensor.matmul(
                                    pt[d][32 * q:32 * (q + 1), g * 4:(g + 1) * 4],
                                    lhsT=whh_sb[d][:, g * 128 + 32 * q:
                                                   g * 128 + 32 * (q + 1)],
                                    rhs=prev[d],
                                    start=False, stop=(g == 3 and q == 3),
                                    tile_position=(0, 32 * q),
                                    skip_group_check=True)
                    th = []
                    for d in range(2):
                        t_ = thpool.tile([128, 16], FP, tag=f"th{d}",
                                         name=f"th{d}_{step}")
                        th.append(t_)
                        nc.scalar.activation(t_[:], pt[d][:, 0:16], AF.Tanh)
                    ab, cn, tcn = [], [], []
                    for d in range(2):
                        ab.append((cellpool.tile([128, BL], FP, tag=f"a{d}",
                                                 name=f"a{d}_{step}"),
                                   cellpool.tile([128, BL], FP, tag=f"b{d}",
                                                 name=f"b{d}_{step}")))
                        cn.append(cstpool.tile([128, BL], FP, tag=f"c{d}",
                                               name=f"c{d}_{step}"))
                        tcn.append(pt[d][:, 16:20])

                    def emit_ab(d):
                        nc.vector.scalar_tensor_tensor(
                            out=ab[d][0][:], in0=th[d][:, 4:8], scalar=1.0,
                            in1=c_cur[d][:], op0=OP.add, op1=OP.mult)
                        nc.vector.scalar_tensor_tensor(
                            out=ab[d][1][:], in0=th[d][:, 0:4], scalar=1.0,
                            in1=th[d][:, 12:16], op0=OP.add, op1=OP.mult)

                    def emit_c(d):
                        nc.vector.scalar_tensor_tensor(
                            out=cn[d][:], in0=ab[d][0][:], scalar=0.5,
                            in1=ab[d][1][:], op0=OP.mult, op1=OP.add)

                    def emit_tc(d):
                        nc.scalar.activation(tcn[d], cn[d][:], AF.Tanh,
                                             scale=0.5)

                    def emit_h(d):
                        nc.vector.scalar_tensor_tensor(
                            out=hs_sb[d][:, tt[d] * BL:(tt[d] + 1) * BL],
                            in0=th[d][:, 8:12], scalar=1.0,
                            in1=tcn[d], op0=OP.add, op1=OP.mult)

                    emit_ab(0)
                    emit_ab(1)
                    emit_c(0)
                    emit_c(1)
                    emit_tc(0)
                    emit_tc(1)
                    emit_h(0)
                    emit_h(1)
                    c_cur = [cn[0], cn[1]]

            # ---- phase 4: emission scores -> d_mid ----
            with tc.tile_pool(name="ps4", bufs=3, space="PSUM") as ps4, \
                 tc.tile_pool(name="fsb", bufs=3) as fpool:
                for ch in range(NTILE):
                    pt = ps4.tile([128, K], FP, space="PSUM")
                    for d in range(2):
                        nc.tensor.matmul(
                            pt[:],
                            lhsT=hs_sb[d][:, ch * 128:(ch + 1) * 128],
                            rhs=wout_sb[d][:],
                            start=(d == 0), stop=(d == 1))
                    fsb = fpool.tile([128, K], FP)
                    nc.vector.tensor_add(fsb[:], pt[:], bout_sb[:])
                    # fsb partition (trh,s,b) -> d_mid row (2ch+trh)*4+b
                    dmv = d_mid.rearrange("(ch2 trh b) (s j) -> ch2 trh s b j",
                                          trh=2, b=BL, j=K)
                    for t in range(2):
                        nc.sync.dma_start(dmv[ch, t], fsb[t * 64:(t + 1) * 64, :])

            # ---- phase 5: chunked Viterbi ----
            # partition p = (c, b): chunk c = t // 16, local s = t % 16
            with tc.tile_pool(name="vit", bufs=1) as vpool, \
                 tc.tile_pool(name="vsc", bufs=2) as spool:
                # feat_cb[p, s*12+j] = feat[b, 16c+s, j]
                feat_cb = vpool.tile([128, LCH * K], FP, name="feat_cb")
                nc.sync.dma_start(feat_cb[:], d_mid)
                # featb[b, (c)*12+j] = feat[b, 16(c+1), j], c = 0..CH-2
                featb = vpool.tile([BL, (CH - 1) * K], FP, name="featb")
                nc.sync.dma_start(
                    featb[:],
                    d_mid.rearrange("(c b) (s j) -> b c s j",
                                    b=BL, j=K)[:, 1:, 0:1, :])

                # --- 5a: within-chunk max-plus matrix prefix products ---
                # G_s[j,k]: slot s at G[:, s*144:(s+1)*144], j-major.
                # G_0 = Id; G_s = M_{16c+s} (.) G_{s-1}
                #   = feat[s,j] + max_m(trans[j,m] + G_{s-1}[m,k])
                G = vpool.tile([128, LCH * KK], FP, name="G")
                nc.vector.tensor_copy(G[:, 0:KK], g0_sb[:])
                tr3 = tr_sb[:].rearrange("p (j m) -> p j m", m=K)
                for s in range(1, LCH):
                    tmp = spool.tile([128, KK * K], FP, tag="vtmp",
                                     name=f"vt{s}")
                    gprev = G[:, (s - 1) * KK:s * KK] \
                        .rearrange("p (m k) -> p m k", k=K) \
                        .unsqueeze(1).transpose([0, 1, 3, 2]) \
                        .broadcast_to([128, K, K, K])
                    nc.vector.tensor_tensor(
                        out=tmp[:].rearrange("p (j k m) -> p j k m", k=K, m=K),
                        in0=tr3.unsqueeze(2).broadcast_to([128, K, K, K]),
                        in1=gprev, op=OP.add)
                    red = spool.tile([128, KK], FP, tag="vred", name=f"vr{s}")
                    nc.vector.reduce_max(
                        red[:], tmp[:].rearrange("p (jk m) -> p jk m", m=K),
                        axis=AX.X)
                    nc.vector.tensor_tensor(
                        out=G[:, s * KK:(s + 1) * KK]
                            .rearrange("p (j k) -> p j k", k=K),
                        in0=red[:].rearrange("p (j k) -> p j k", k=K),
                        in1=feat_cb[:, s * K:(s + 1) * K].unsqueeze(2)
                            .broadcast_to([128, K, K]),
                        op=OP.add)

                # --- 5b: cross-chunk serial scan (partitions 0..3) ---
                # v_c = scores at t=16c; u_c = scores at t=16c-1
                nc.sync.dma_start(d_g15, G[:, (LCH - 1) * KK:])
                gb = vpool.tile([BL, CH * KK], FP, name="gb")
                nc.sync.dma_start(
                    gb[:], d_g15.rearrange("(c b) jk -> b c jk", b=BL))
                uall = vpool.tile([BL, CH * K], FP, name="uall")
                vall = vpool.tile([BL, CH * K], FP, name="vall")
                sfin = vpool.tile([BL, K], FP, name="sfin")
                nc.vector.tensor_copy(uall[:, 0:K], s0_sb[:])
                # v_0[j] = trans[j,START] + feat_0[j]  (scores0[START]=0)
                nc.vector.tensor_tensor(
                    out=vall[:, 0:K], in0=feat_cb[0:BL, 0:K], in1=tr9_sb[:],
                    op=OP.add)
                tr3b = tr_sb[0:BL, :].rearrange("p (j m) -> p j m", m=K)
                for c in range(CH):
                    tb = spool.tile([BL, KK], FP, tag="btmp", name=f"bt{c}")
                    nc.vector.tensor_tensor(
                        out=tb[:].rearrange("p (j k) -> p j k", k=K),
                        in0=gb[:, c * KK:(c + 1) * KK]
                            .rearrange("p (j k) -> p j k", k=K),
                        in1=vall[:, c * K:(c + 1) * K].unsqueeze(1)
                            .broadcast_to([BL, K, K]),
                        op=OP.add)
                    utgt = uall[:, (c + 1) * K:(c + 2) * K] if c < CH - 1 \
                        else sfin[:]
                    nc.vector.reduce_max(
                        utgt, tb[:].rearrange("p (j k) -> p j k", k=K),
                        axis=AX.X)
                    if c < CH - 1:
                        tb2 = spool.tile([BL, KK], FP, tag="btm2",
                                         name=f"b2{c}")
                        nc.vector.tensor_tensor(
                            out=tb2[:].rearrange("p (j m) -> p j m", m=K),
                            in0=tr3b,
                            in1=uall[:, (c + 1) * K:(c + 2) * K].unsqueeze(1)
                                .broadcast_to([BL, K, K]),
                            op=OP.add)
                        red4 = spool.tile([BL, K], FP, tag="bred",
                                          name=f"br{c}")
                        nc.vector.reduce_max(
                            red4[:],
                            tb2[:].rearrange("p (j m) -> p j m", m=K),
                            axis=AX.X)
                        nc.vector.tensor_tensor(
                            out=vall[:, (c + 1) * K:(c + 2) * K],
                            in0=red4[:], in1=featb[:, c * K:(c + 1) * K],
                            op=OP.add)

                # --- 5c: batched prefix application: s_t = G_s (.) v_c ---
                nc.sync.dma_start(d_u, uall[:])
                nc.sync.dma_start(d_v, vall[:])
                u128 = vpool.tile([128, K], FP, name="u128")
                v128 = vpool.tile([128, K], FP, name="v128")
                nc.sync.dma_start(
                    u128[:], d_u.rearrange("b (c j) -> c b j", j=K))
                nc.sync.dma_start(
                    v128[:], d_v.rearrange("b (c j) -> c b j", j=K))
                tmp2 = spool.tile([128, LCH * KK], FP, tag="vap", name="vap")
                nc.vector.tensor_tensor(
                    out=tmp2[:].rearrange("p (s j k) -> p s j k", j=K, k=K),
                    in0=G[:].rearrange("p (s j k) -> p s j k", j=K, k=K),
                    in1=v128[:].unsqueeze(1).unsqueeze(1)
                        .broadcast_to([128, LCH, K, K]),
                    op=OP.add)
                s_all = vpool.tile([128, LCH * K], FP, name="s_all")
                nc.vector.reduce_max(
                    s_all[:], tmp2[:].rearrange("p (sj k) -> p sj k", k=K),
                    axis=AX.X)
                # sprev[p, s, :] = scores at t = 16c+s-1
                sprev = vpool.tile([128, LCH * K], FP, name="sprev")
                nc.vector.tensor_copy(sprev[:, K:], s_all[:, 0:(LCH - 1) * K])
                nc.vector.tensor_copy(sprev[:, 0:K], u128[:])

                # --- 5d: pointer extraction (argmax prev tag, w-encoded) ---
                wptr = vpool.tile([128, LCH * K], FP, name="wptr")
                w3 = wptr[:].rearrange("p (s j) -> p s j", j=K)
                sp3 = sprev[:].rearrange("p (s k) -> p s k", k=K)
                for j in range(K):
                    madd = spool.tile([128, LCH * K], FP, tag="pm",
                                      name=f"pm{j}")
                    m3 = madd[:].rearrange("p (s k) -> p s k", k=K)
                    nc.vector.tensor_tensor(
                        out=m3, in0=sp3,
                        in1=tr_sb[:, j * K:(j + 1) * K].unsqueeze(1)
                            .broadcast_to([128, LCH, K]),
                        op=OP.add)
                    mxj = spool.tile([128, LCH], FP, tag="px", name=f"px{j}")
                    nc.vector.reduce_max(mxj[:], m3, axis=AX.X)
                    msk = spool.tile([128, LCH * K], FP, tag="pk",
                                     name=f"pk{j}")
                    mk3 = msk[:].rearrange("p (s k) -> p s k", k=K)
                    nc.vector.tensor_tensor(
                        out=mk3, in0=m3,
                        in1=mxj[:].unsqueeze(2).broadcast_to([128, LCH, K]),
                        op=OP.is_equal)
                    nc.vector.tensor_tensor(
                        out=mk3, in0=mk3,
                        in1=wv_sb[:].unsqueeze(1).broadcast_to([128, LCH, K]),
                        op=OP.mult)
                    nc.vector.reduce_max(w3[:, :, j], mk3, axis=AX.X)
                # decode w -> plain previous-tag table F_s(x) = ptd[p, s*12+x]
                ptd = vpool.tile([128, LCH * K], FP, name="ptd")
                nc.vector.tensor_scalar(
                    out=ptd[:], in0=wptr[:], scalar1=-1.0, scalar2=float(K - 1),
                    op0=OP.mult, op1=OP.add)

                # --- 5e: backtrace map composition C_s = F_s o C_{s+1} ---
                Call = vpool.tile([128, LCH * K], FP, name="Call")
                nc.vector.tensor_copy(
                    Call[:, (LCH - 1) * K:], ptd[:, (LCH - 1) * K:])
                for s in range(LCH - 2, -1, -1):
                    oh = spool.tile([128, KK], FP, tag="co", name=f"co{s}")
                    oh3 = oh[:].rearrange("p (x y) -> p x y", y=K)
                    nc.vector.tensor_tensor(
                        out=oh3,
                        in0=Call[:, (s + 1) * K:(s + 2) * K].unsqueeze(2)
                            .broadcast_to([128, K, K]),
                        in1=io_sb[:].unsqueeze(1).broadcast_to([128, K, K]),
                        op=OP.is_equal)
                    nc.vector.tensor_tensor(
                        out=oh3, in0=oh3,
                        in1=ptd[:, s * K:(s + 1) * K].unsqueeze(1)
                            .broadcast_to([128, K, K]),
                        op=OP.mult)
                    nc.vector.reduce_max(
                        Call[:, s * K:(s + 1) * K], oh3, axis=AX.X)

                # --- 5f: init best tag (t = T-1) ---
                fs = spool.tile([BL, K], FP, tag="bt1", name="fs")
                nc.vector.tensor_add(fs[:], sfin[:], ts_sb[:])
                mx8 = spool.tile([BL, 8], FP, tag="mx8", name="mx8")
                nc.vector.max(mx8[:], fs[:])
                msk4 = spool.tile([BL, K], FP, tag="bmsk", name="msk4")
                nc.vector.tensor_scalar(
                    out=msk4[:], in0=fs[:], scalar1=mx8[:, 0:1], scalar2=None,
                    op0=OP.is_equal)
                nc.vector.tensor_mul(msk4[:], msk4[:], wv_sb[0:BL, :])
                w4 = spool.tile([BL, 1], FP, tag="w4", name="w4")
                nc.vector.reduce_max(w4[:], msk4[:], axis=AX.X)

                # --- 5g: serial chunk-boundary backtrace (x_c = end tag) ---
                nc.sync.dma_start(d_c0, Call[:, 0:K])
                c0b = vpool.tile([BL, CH * K], FP, name="c0b")
                nc.sync.dma_start(
                    c0b[:], d_c0.rearrange("(c b) j -> b c j", b=BL))
                xall = vpool.tile([BL, CH], FP, name="xall")
                nc.vector.tensor_scalar(
                    out=xall[:, CH - 1:CH], in0=w4[:], scalar1=-1.0,
                    scalar2=float(K - 1), op0=OP.mult, op1=OP.add)
                for c in range(CH - 1, 0, -1):
                    oh4 = spool.tile([BL, K], FP, tag="bo", name=f"bo{c}")
                    nc.vector.tensor_scalar(
                        out=oh4[:], in0=io_sb[0:BL, :],
                        scalar1=xall[:, c:c + 1], scalar2=None,
                        op0=OP.is_equal)
                    nc.vector.tensor_mul(
                        oh4[:], oh4[:], c0b[:, c * K:(c + 1) * K])
                    nc.vector.reduce_max(xall[:, c - 1:c], oh4[:], axis=AX.X)

                # --- 5h: batched tag application ---
                nc.sync.dma_start(d_x, xall[:])
                x128 = vpool.tile([128, 1], FP, name="x128")
                nc.sync.dma_start(x128[:], d_x.rearrange("b c -> c b"))
                oh128 = vpool.tile([128, K], FP, name="oh128")
                nc.vector.tensor_scalar(
                    out=oh128[:], in0=io_sb[:], scalar1=x128[:, 0:1],
                    scalar2=None, op0=OP.is_equal)
                tgt = spool.tile([128, LCH * K], FP, tag="tg", name="tgt")
                t3 = tgt[:].rearrange("p (s y) -> p s y", y=K)
                nc.vector.tensor_tensor(
                    out=t3,
                    in0=Call[:].rearrange("p (s y) -> p s y", y=K),
                    in1=oh128[:].unsqueeze(1).broadcast_to([128, LCH, K]),
                    op=OP.mult)
                tags = vpool.tile([128, LCH], FP, name="tags")
                nc.vector.reduce_max(tags[:], t3, axis=AX.X)
                pi_cb = vpool.tile([128, LCH], I32, name="pi_cb")
                nc.vector.tensor_copy(pi_cb[:], tags[:])
                xi = vpool.tile([BL, 1], I32, name="xi")
                nc.vector.tensor_copy(xi[:], xall[:, CH - 1:CH])

                # --- 5i: assemble path [b, t] ---
                # tags[p=(c,b), s] is the tag at t = 16c+s-1
                nc.sync.dma_start(
                    d_tags.rearrange("c b s -> (c b) s"), pi_cb[:])
                nc.sync.dma_start(
                    path_out[:, 0:LCH - 1], d_tags[0, :, 1:LCH])
                nc.sync.dma_start(
                    path_out[:, LCH - 1:T - 1],
                    d_tags.rearrange("c b s -> b c s")[:, 1:, :])
                nc.sync.dma_start(path_out[:, T - 1:T], xi[:])

    nc.compile()
    return nc


def prep_inputs(sentence, h0, c0, embed, W_ih_f, W_hh_f, b_f, W_ih_r, W_hh_r,
                b_r, W_out, b_out, transitions, T=512):
    """Host-side layout prep. Returns per-core input maps."""
    f32 = np.float32
    perm = np.r_[0:128, 128:256, 384:512, 256:384]  # i,f,g,o -> i,f,o,g
    gs = np.concatenate([np.full(128, s, f32) for s in (0.5, 0.5, 0.5, 1.0)])

    def prep_dir(W_ih, W_hh, b):
        Wi = np.asarray(W_ih, f32)[perm] * gs[:, None]
        bb = np.asarray(b, f32)[perm] * gs
        Wh = np.asarray(W_hh, f32)[perm] * (0.5 * gs)[:, None]
        return Wi.T.copy(), Wh.T.copy(), bb

    wihT_f, whhT_f, be_f = prep_dir(W_ih_f, W_hh_f, b_f)
    wihT_r, whhT_r, be_r = prep_dir(W_ih_r, W_hh_r, b_r)
    w_ihT = np.stack([wihT_f, wihT_r])
    w_hhT = np.stack([whhT_f, whhT_r]).astype(np.float16)
    b_in = np.stack([be_f.reshape(4, 128), be_r.reshape(4, 128)])  # [2,4,128]
    b_in = b_in.reshape(8, 128).T.copy()                           # [128,8]

    Wo = np.asarray(W_out, f32) * 0.5
    w_outT = np.stack([Wo[:, :128].T.copy(), Wo[:, 128:].T.copy()]).astype(np.float16)
    bout_rep = np.tile(np.asarray(b_out, f32)[None, :], (128, 1))

    tr = np.asarray(transitions, f32)
    trans128 = np.tile(tr.reshape(1, K * K), (128, 1))
    wvec128 = np.tile((K - 1 - np.arange(K, dtype=f32))[None, :], (128, 1))
    tstop = np.tile(tr[STOP][None, :], (BL, 1))
    s0 = np.full((BL, K), NEG, f32)
    s0[:, START] = 0.0
    ident = np.eye(128, dtype=f32)
    g0 = np.full((K, K), NEG, f32)
    np.fill_diagonal(g0, 0.0)
    g0_in = np.tile(g0.reshape(1, K * K), (128, 1))
    tr9_in = np.tile(tr[:, START][None, :], (BL, 1))
    iota_in = np.tile(np.arange(K, dtype=f32)[None, :], (128, 1))
    embed = np.asarray(embed, f32)
    sentence = np.asarray(sentence)

    maps = []
    for core in range(NCORES):
        sl = sentence[core * BL:(core + 1) * BL, :T].astype(np.int32)
        idx_tm = sl.T.reshape(-1)                       # n = t*BL+b
        idx_in = idx_tm.reshape(-1, 128).T.copy()       # [128, NTILE]
        h_i = 2.0 * np.asarray(h0, f32)[:, core * BL:(core + 1) * BL, :]
        c_i = 2.0 * np.asarray(c0, f32)[:, core * BL:(core + 1) * BL, :]
        maps.append({
            "idx_in": idx_in,
            "embed": embed,
            "w_ihT": w_ihT,
            "w_hhT": w_hhT,
            "b_in": b_in,
            "h_init": np.ascontiguousarray(h_i.transpose(0, 2, 1)).astype(np.float16),
            "c_init": np.ascontiguousarray(c_i.transpose(0, 2, 1)),
            "w_outT": w_outT,
            "bout_rep": bout_rep,
            "ident": ident,
            "identr": ident.astype(np.float16),
            "trans128": trans128,
            "wvec128": wvec128,
            "tstop": tstop,
            "scores0": s0,
            "g0_in": g0_in,
            "tr9_in": tr9_in,
            "iota_in": iota_in,
        })
    return maps


_NC_CACHE = {}


def kernel(sentence, h0, c0, embed, W_ih_f, W_hh_f, b_f, W_ih_r, W_hh_r, b_r,
           W_out, b_out, transitions):
    T = np.asarray(sentence).shape[1]
    if T not in _NC_CACHE:
        _NC_CACHE[T] = build_program(T)
    nc = _NC_CACHE[T]
    maps = prep_inputs(sentence, h0, c0, embed, W_ih_f, W_hh_f, b_f,
                       W_ih_r, W_hh_r, b_r, W_out, b_out, transitions, T=T)
    res = run_bass_kernel_spmd(nc, maps, list(range(NCORES)))
    out = np.concatenate([res.results[i]["path_out"] for i in range(NCORES)], axis=0)
    return out.astype(np.int32)


# revision 21
# speedup vs baseline: 1.0145x; 1.0145x over previous
"""BiLSTM-CRF Trainium2 kernel (8-core SPMD, batch-sharded).

Per core: 4 sequences, full pipeline on device:
  embedding gather (indirect DMA) -> PE transposes -> input-gate GEMMs ->
  512-step bidirectional LSTM recurrence -> emission GEMM ->
  chunked max-plus Viterbi scan -> batched pointer extraction ->
  chunked backtrace via composed pointer maps -> int32 tag path.

Key performance structure vs the naive formulation:
  * The input GEMMs use float32r (single PE pass, full-rate with a 512
    moving dim); the LSTM recurrence uses float16 operands (same 10-bit
    mantissa as tf32) in 32-col quadrant matmuls whose weight loads
    pipeline across PE sub-array strips, halving the fp32 2-pass cost.
  * The Viterbi scan is chunked: 32 chunks x 16 steps. Within-chunk
    max-plus matrix prefix products are computed with all 128 chunks
    (chunk, seq) batched on partitions (15 serial steps instead of 512),
    followed by a tiny serial cross-chunk vector scan and a batched
    prefix application.
  * The backtrace composes pointer maps the same way: within-chunk map
    composition (one-hot gather algebra), serial chunk-boundary
    evaluation, then a batched apply.

Math notes:
  sigmoid(x) = 0.5*tanh(0.5x)+0.5 so every gate uses one Tanh activation;
  the 0.5 factors are pre-folded into the weights. Cell/hidden state are
  carried doubled (C=2c, H=2h); the 0.5 for H is folded into W_hh/W_out.
"""

import numpy as np

import concourse.bass as bass
import concourse.tile as tile
from concourse import bacc, mybir
from concourse.bass_utils import run_bass_kernel_spmd

FP = mybir.dt.float32
FPR = mybir.dt.float32r
F16 = mybir.dt.float16
I32 = mybir.dt.int32
AX = mybir.AxisListType
OP = mybir.AluOpType
AF = mybir.ActivationFunctionType

VOCAB = 100000
E = 256
Hh = 128
K = 12
START = 9
STOP = 10
NEG = -10000.0
B = 32
NCORES = 8
BL = B // NCORES  # 4 sequences per core
LCH = 16          # Viterbi chunk length
KK = K * K


def build_program(T=512):
    nc = bacc.Bacc("TRN2", target_bir_lowering=False, debug=False)
    NTOK = T * BL              # tokens per core
    NTILE = NTOK // 128        # gather tiles (16 at T=512)
    NCHUNK = NTOK // 512       # 512-col GEMM chunks (4)
    CH = T // LCH              # Viterbi chunks (32); CH*BL == 128

    def din(name, shape, dtype=FP):
        return nc.dram_tensor(name, list(shape), dtype, kind="ExternalInput").ap()

    idx_in = din("idx_in", [128, NTILE], I32)          # [p,k] token ids, time-major
    embed = din("embed", [VOCAB, E])
    w_ihT = din("w_ihT", [2, E, 4 * Hh], FPR)               # pre-scaled, gate order i,f,o,g
    w_hhT = din("w_hhT", [2, Hh, 4 * Hh], F16)
    b_in = din("b_in", [128, 8])                       # col d*4+g: per-partition bias
    h_init = din("h_init", [2, 128, BL], F16)               # 2*h0, feature-major
    c_init = din("c_init", [2, 128, BL])               # 2*c0
    w_outT = din("w_outT", [2, Hh, K], F16)                 # 0.5*W_out halves, transposed
    bout_rep = din("bout_rep", [128, K])
    ident = din("ident", [128, 128])
    identr = din("identr", [128, 128], F16)
    trans128 = din("trans128", [128, KK])              # trans[j,m] flat, replicated
    wvec128 = din("wvec128", [128, K])                 # 11-k, replicated
    tstop = din("tstop", [BL, K])                      # trans[STOP,:] replicated
    scores0 = din("scores0", [BL, K])
    g0_in = din("g0_in", [128, KK])                    # max-plus identity, replicated
    tr9_in = din("tr9_in", [BL, K])                    # trans[:,START] replicated
    iota_in = din("iota_in", [128, K])                 # 0..11 replicated

    path_out = nc.dram_tensor("path_out", [BL, T], I32, kind="ExternalOutput").ap()

    # DRAM scratch for partition-permute bounces
    d_mid = nc.dram_tensor("d_mid", [128, LCH * K], FP).ap()
    d_g15 = nc.dram_tensor("d_g15", [128, KK], FP).ap()
    d_u = nc.dram_tensor("d_u", [BL, CH * K], FP).ap()
    d_v = nc.dram_tensor("d_v", [BL, CH * K], FP).ap()
    d_c0 = nc.dram_tensor("d_c0", [128, K], FP).ap()
    d_x = nc.dram_tensor("d_x", [BL, CH], FP).ap()
    d_tags = nc.dram_tensor("d_tags", [CH, BL, LCH], I32).ap()

    with tile.TileContext(nc) as tc:
        with tc.tile_pool(name="const", bufs=1) as cpool, \
             tc.tile_pool(name="big", bufs=1) as bpool:

            # ---- load constants ----
            def cload(ap_in, shape, dtype=FP):
                t = cpool.tile(list(shape), dtype, name=f"c_{np.random.randint(1<<30)}")
                nc.sync.dma_start(t[:], ap_in)
                return t

            idx_sb = cload(idx_in, [128, NTILE], I32)
            wih_sb = [[cload(w_ihT[d, e * 128:(e + 1) * 128, :], [128, 4 * Hh], FPR)
                       for e in range(2)] for d in range(2)]
            whh_sb = [cload(w_hhT[d], [Hh, 4 * Hh], F16) for d in range(2)]
            b_sb = cload(b_in, [128, 8])
            hi_sb = [cload(h_init[d], [128, BL], F16) for d in range(2)]
            ci_sb = [cload(c_init[d], [128, BL]) for d in range(2)]
            wout_sb = [cload(w_outT[d], [Hh, K], F16) for d in range(2)]
            bout_sb = cload(bout_rep, [128, K])
            id_sb = cload(ident, [128, 128])
            idr_sb = cload(identr, [128, 128], F16)
            tr_sb = cload(trans128, [128, KK])
            wv_sb = cload(wvec128, [128, K])
            ts_sb = cload(tstop, [BL, K])
            s0_sb = cload(scores0, [BL, K])
            g0_sb = cload(g0_in, [128, KK])
            tr9_sb = cload(tr9_in, [BL, K])
            io_sb = cload(iota_in, [128, K])

            # big persistent arrays
            xg_sb = [bpool.tile([128, T * 16], F16, tag=f"xg{d}", name=f"xg{d}") for d in range(2)]
            hs_sb = [bpool.tile([128, T * BL], F16, tag=f"hs{d}", name=f"hs{d}") for d in range(2)]

            # ---- phase 1: embedding gather + transpose to [E, tok] ----
            with tc.tile_pool(name="gat", bufs=3) as gpool, \
                 tc.tile_pool(name="ps1", bufs=4, space="PSUM") as ps1, \
                 tc.tile_pool(name="xe", bufs=1) as xepool:
                xe_sb = [xepool.tile([128, NTOK], FPR, tag=f"xe{e}", name=f"xe{e}") for e in range(2)]
                for k in range(NTILE):
                    gt = gpool.tile([128, E], FP)
                    nc.gpsimd.indirect_dma_start(
                        out=gt[:],
                        out_offset=None,
                        in_=embed[:],
                        in_offset=bass.IndirectOffsetOnAxis(
                            ap=idx_sb[:, k:k + 1], axis=0),
                    )
                    for e in range(2):
                        pt = ps1.tile([128, 128], FP, space="PSUM")
                        nc.tensor.transpose(
                            out=pt[:], in_=gt[:, e * 128:(e + 1) * 128],
                            identity=id_sb[:])
                        nc.vector.tensor_copy(
                            xe_sb[e][:, k * 128:(k + 1) * 128], pt[:])

                # ---- phase 2: xg = W_ih_eff @ xe + b, interleaved [t,(g,b)] ----
                with tc.tile_pool(name="ps2", bufs=3, space="PSUM") as ps2:
                    for d in range(2):
                        xgv = xg_sb[d][:].rearrange("p (t x) -> p t x", x=16)
                        for g in range(4):
                            for c in range(NCHUNK):
                                pt = ps2.tile([128, 512], FP, space="PSUM")
                                for e in range(2):
                                    nc.tensor.matmul(
                                        pt[:],
                                        lhsT=wih_sb[d][e][:, g * 128:(g + 1) * 128],
                                        rhs=xe_sb[e][:, c * 512:(c + 1) * 512],
                                        start=(e == 0), stop=(e == 1),
                                    )
                                nc.vector.tensor_scalar(
                                    out=xgv[:, c * 128:(c + 1) * 128,
                                            g * 4:(g + 1) * 4],
                                    in0=pt[:].rearrange("p (t b) -> p t b", b=BL),
                                    scalar1=b_sb[:, d * 4 + g:d * 4 + g + 1],
                                    scalar2=None,
                                    op0=OP.add,
                                )

            # ---- phase 3: LSTM recurrence, two staggered chains ----
            # gate cols per step: i=0:4, f=4:8, o=8:12, g=12:16
            # Emission order is chosen so each engine's FIFO matches the
            # arrival order of a half-step-staggered steady state:
            # PE [inj0 whh0 inj1 whh1], ACT [tanh0 tanh1 tc0 tc1],
            # DVE [a0 b0 c0 a1 h0 b1 c1 h1].
            with tc.tile_pool(name="ps3", bufs=4, space="PSUM") as ps3, \
                 tc.tile_pool(name="th", bufs=4) as thpool, \
                 tc.tile_pool(name="cell", bufs=4) as cellpool, \
                 tc.tile_pool(name="cst", bufs=2) as cstpool:
                c_cur = [ci_sb[0], ci_sb[1]]
                for step in range(T):
                    tt = [step, T - 1 - step]
                    prev = [hi_sb[d][:] if step == 0 else
                            hs_sb[d][:, (tt[d] - 1 + 2 * d) * BL:
                                      (tt[d] + 2 * d) * BL]
                            for d in range(2)]
                    pt = []
                    for d in range(2):
                        p = ps3.tile([128, 16], FP, space="PSUM",
                                     tag=f"g{d}", name=f"g{d}_{step}")
                        pt.append(p)
                        for q in range(4):
                            nc.tensor.matmul(
                                p[32 * q:32 * (q + 1), :],
                                lhsT=idr_sb[:, 32 * q:32 * (q + 1)],
                                rhs=xg_sb[d][:, tt[d] * 16:(tt[d] + 1) * 16],
                                start=True, stop=False,
                                tile_position=(0, 32 * q),
                                skip_group_check=True)
                    for d in range(2):
                        for g in range(4):
                            for q in range(4):
                                nc.tensor.matmul(
                                    pt[d][32 * q:32 * (q + 1), g * 4:(g + 1) * 4],
                                    lhsT=whh_sb[d][:, g * 128 + 32 * q:
                                                   g * 128 + 32 * (q + 1)],
                                    rhs=prev[d],
                                    start=False, stop=(g == 3 and q == 3),
                                    tile_position=(0, 32 * q),
                                    skip_group_check=True)
                    th = []
                    for d in range(2):
                        t_ = thpool.tile([128, 16], FP, tag=f"th{d}",
                                         name=f"th{d}_{step}")
                        th.append(t_)
                        nc.scalar.activation(t_[:], pt[d][:], AF.Tanh)
                    ab, cn, tcn = [], [], []
                    for d in range(2):
                        ab.append((cellpool.tile([128, BL], FP, tag=f"a{d}",
                                                 name=f"a{d}_{step}"),
                                   cellpool.tile([128, BL], FP, tag=f"b{d}",
                                                 name=f"b{d}_{step}")))
                        cn.append(cstpool.tile([128, BL], FP, tag=f"c{d}",
                                               name=f"c{d}_{step}"))
                        tcn.append(cellpool.tile([128, BL], FP, tag=f"tc{d}",
                                                 name=f"tc{d}_{step}"))

                    def emit_ab(d):
                        nc.vector.scalar_tensor_tensor(
                            out=ab[d][0][:], in0=th[d][:, 4:8], scalar=1.0,
                            in1=c_cur[d][:], op0=OP.add, op1=OP.mult)
                        nc.vector.scalar_tensor_tensor(
                            out=ab[d][1][:], in0=th[d][:, 0:4], scalar=1.0,
                            in1=th[d][:, 12:16], op0=OP.add, op1=OP.mult)

                    def emit_c(d):
                        nc.vector.scalar_tensor_tensor(
                            out=cn[d][:], in0=ab[d][0][:], scalar=0.5,
                            in1=ab[d][1][:], op0=OP.mult, op1=OP.add)

                    def emit_tc(d):
                        nc.scalar.activation(tcn[d][:], cn[d][:], AF.Tanh,
                                             scale=0.5)

                    def emit_h(d):
                        nc.vector.scalar_tensor_tensor(
                            out=hs_sb[d][:, tt[d] * BL:(tt[d] + 1) * BL],
                            in0=th[d][:, 8:12], scalar=1.0,
                            in1=tcn[d][:], op0=OP.add, op1=OP.mult)

                    emit_ab(0)
                    emit_ab(1)
                    emit_c(0)
                    emit_c(1)
                    emit_tc(0)
                    emit_tc(1)
                    emit_h(0)
                    emit_h(1)
                    c_cur = [cn[0], cn[1]]

            # ---- phase 4: emission scores -> d_mid ----
            with tc.tile_pool(name="ps4", bufs=3, space="PSUM") as ps4, \
                 tc.tile_pool(name="fsb", bufs=3) as fpool:
                for ch in range(NTILE):
                    pt = ps4.tile([128, K], FP, space="PSUM")
                    for d in range(2):
                        nc.tensor.matmul(
                            pt[:],
                            lhsT=hs_sb[d][:, ch * 128:(ch + 1) * 128],
                            rhs=wout_sb[d][:],
                            start=(d == 0), stop=(d == 1))
                    fsb = fpool.tile([128, K], FP)
                    nc.vector.tensor_add(fsb[:], pt[:], bout_sb[:])
                    # fsb partition (trh,s,b) -> d_mid row (2ch+trh)*4+b
                    dmv = d_mid.rearrange("(ch2 trh b) (s j) -> ch2 trh s b j",
                                          trh=2, b=BL, j=K)
                    for t in range(2):
                        nc.sync.dma_start(dmv[ch, t], fsb[t * 64:(t + 1) * 64, :])

            # ---- phase 5: chunked Viterbi ----
            # partition p = (c, b): chunk c = t // 16, local s = t % 16
            with tc.tile_pool(name="vit", bufs=1) as vpool, \
                 tc.tile_pool(name="vsc", bufs=2) as spool:
                # feat_cb[p, s*12+j] = feat[b, 16c+s, j]
                feat_cb = vpool.tile([128, LCH * K], FP, name="feat_cb")
                nc.sync.dma_start(feat_cb[:], d_mid)
                # featb[b, (c)*12+j] = feat[b, 16(c+1), j], c = 0..CH-2
                featb = vpool.tile([BL, (CH - 1) * K], FP, name="featb")
                nc.sync.dma_start(
                    featb[:],
                    d_mid.rearrange("(c b) (s j) -> b c s j",
                                    b=BL, j=K)[:, 1:, 0:1, :])

                # --- 5a: within-chunk max-plus matrix prefix products ---
                # G_s[j,k]: slot s at G[:, s*144:(s+1)*144], j-major.
                # G_0 = Id; G_s = M_{16c+s} (.) G_{s-1}
                #   = feat[s,j] + max_m(trans[j,m] + G_{s-1}[m,k])
                G = vpool.tile([128, LCH * KK], FP, name="G")
                nc.vector.tensor_copy(G[:, 0:KK], g0_sb[:])
                tr3 = tr_sb[:].rearrange("p (j m) -> p j m", m=K)
                for s in range(1, LCH):
                    tmp = spool.tile([128, KK * K], FP, tag="vtmp",
                                     name=f"vt{s}")
                    gprev = G[:, (s - 1) * KK:s * KK] \
                        .rearrange("p (m k) -> p m k", k=K) \
                        .unsqueeze(1).transpose([0, 1, 3, 2]) \
                        .broadcast_to([128, K, K, K])
                    nc.vector.tensor_tensor(
                        out=tmp[:].rearrange("p (j k m) -> p j k m", k=K, m=K),
                        in0=tr3.unsqueeze(2).broadcast_to([128, K, K, K]),
                        in1=gprev, op=OP.add)
                    red = spool.tile([128, KK], FP, tag="vred", name=f"vr{s}")
                    nc.vector.reduce_max(
                        red[:], tmp[:].rearrange("p (jk m) -> p jk m", m=K),
                        axis=AX.X)
                    nc.vector.tensor_tensor(
                        out=G[:, s * KK:(s + 1) * KK]
                            .rearrange("p (j k) -> p j k", k=K),
                        in0=red[:].rearrange("p (j k) -> p j k", k=K),
                        in1=feat_cb[:, s * K:(s + 1) * K].unsqueeze(2)
                            .broadcast_to([128, K, K]),
                        op=OP.add)

                # --- 5b: cross-chunk serial scan (partitions 0..3) ---
                # v_c = scores at t=16c; u_c = scores at t=16c-1
                nc.sync.dma_start(d_g15, G[:, (LCH - 1) * KK:])
                gb = vpool.tile([BL, CH * KK], FP, name="gb")
                nc.sync.dma_start(
                    gb[:], d_g15.rearrange("(c b) jk -> b c jk", b=BL))
                uall = vpool.tile([BL, CH * K], FP, name="uall")
                vall = vpool.tile([BL, CH * K], FP, name="vall")
                sfin = vpool.tile([BL, K], FP, name="sfin")
                nc.vector.tensor_copy(uall[:, 0:K], s0_sb[:])
                # v_0[j] = trans[j,START] + feat_0[j]  (scores0[START]=0)
                nc.vector.tensor_tensor(
                    out=vall[:, 0:K], in0=feat_cb[0:BL, 0:K], in1=tr9_sb[:],
                    op=OP.add)
                tr3b = tr_sb[0:BL, :].rearrange("p (j m) -> p j m", m=K)
                for c in range(CH):
                    tb = spool.tile([BL, KK], FP, tag="btmp", name=f"bt{c}")
                    nc.vector.tensor_tensor(
                        out=tb[:].rearrange("p (j k) -> p j k", k=K),
                        in0=gb[:, c * KK:(c + 1) * KK]
                            .rearrange("p (j k) -> p j k", k=K),
                        in1=vall[:, c * K:(c + 1) * K].unsqueeze(1)
                            .broadcast_to([BL, K, K]),
                        op=OP.add)
                    utgt = uall[:, (c + 1) * K:(c + 2) * K] if c < CH - 1 \
                        else sfin[:]
                    nc.vector.reduce_max(
                        utgt, tb[:].rearrange("p (j k) -> p j k", k=K),
                        axis=AX.X)
                    if c < CH - 1:
                        tb2 = spool.tile([BL, KK], FP, tag="btm2",
                                         name=f"b2{c}")
                        nc.vector.tensor_tensor(
                            out=tb2[:].rearrange("p (j m) -> p j m", m=K),
                            in0=tr3b,
                            in1=uall[:, (c + 1) * K:(c + 2) * K].unsqueeze(1)
                                .broadcast_to([BL, K, K]),
                            op=OP.add)
                        red4 = spool.tile([BL, K], FP, tag="bred",
                                          name=f"br{c}")
                        nc.vector.reduce_max(
                            red4[:],
                            tb2[:].rearrange("p (j m) -> p j m", m=K),
                            axis=AX.X)
                        nc.vector.tensor_tensor(
                            out=vall[:, (c + 1) * K:(c + 2) * K],
                            in0=red4[:], in1=featb[:, c * K:(c + 1) * K],
                            op=OP.add)

                # --- 5c: batched prefix application: s_t = G_s (.) v_c ---
                nc.sync.dma_start(d_u, uall[:])
                nc.sync.dma_start(d_v, vall[:])
                u128 = vpool.tile([128, K], FP, name="u128")
                v128 = vpool.tile([128, K], FP, name="v128")
                nc.sync.dma_start(
                    u128[:], d_u.rearrange("b (c j) -> c b j", j=K))
                nc.sync.dma_start(
                    v128[:], d_v.rearrange("b (c j) -> c b j", j=K))
                tmp2 = spool.tile([128, LCH * KK], FP, tag="vap", name="vap")
                nc.vector.tensor_tensor(
                    out=tmp2[:].rearrange("p (s j k) -> p s j k", j=K, k=K),
                    in0=G[:].rearrange("p (s j k) -> p s j k", j=K, k=K),
                    in1=v128[:].unsqueeze(1).unsqueeze(1)
                        .broadcast_to([128, LCH, K, K]),
                    op=OP.add)
                s_all = vpool.tile([128, LCH * K], FP, name="s_all")
                nc.vector.reduce_max(
                    s_all[:], tmp2[:].rearrange("p (sj k) -> p sj k", k=K),
                    axis=AX.X)
                # sprev[p, s, :] = scores at t = 16c+s-1
                sprev = vpool.tile([128, LCH * K], FP, name="sprev")
                nc.vector.tensor_copy(sprev[:, K:], s_all[:, 0:(LCH - 1) * K])
                nc.vector.tensor_copy(sprev[:, 0:K], u128[:])

                # --- 5d: pointer extraction (argmax prev tag, w-encoded) ---
                wptr = vpool.tile([128, LCH * K], FP, name="wptr")
                w3 = wptr[:].rearrange("p (s j) -> p s j", j=K)
                sp3 = sprev[:].rearrange("p (s k) -> p s k", k=K)
                for j in range(K):
                    madd = spool.tile([128, LCH * K], FP, tag="pm",
                                      name=f"pm{j}")
                    m3 = madd[:].rearrange("p (s k) -> p s k", k=K)
                    nc.vector.tensor_tensor(
                        out=m3, in0=sp3,
                        in1=tr_sb[:, j * K:(j + 1) * K].unsqueeze(1)
                            .broadcast_to([128, LCH, K]),
                        op=OP.add)
                    mxj = spool.tile([128, LCH], FP, tag="px", name=f"px{j}")
                    nc.vector.reduce_max(mxj[:], m3, axis=AX.X)
                    msk = spool.tile([128, LCH * K], FP, tag="pk",
                                     name=f"pk{j}")
                    mk3 = msk[:].rearrange("p (s k) -> p s k", k=K)
                    nc.vector.tensor_tensor(
                        out=mk3, in0=m3,
                        in1=mxj[:].unsqueeze(2).broadcast_to([128, LCH, K]),
                        op=OP.is_equal)
                    nc.vector.tensor_tensor(
                        out=mk3, in0=mk3,
                        in1=wv_sb[:].unsqueeze(1).broadcast_to([128, LCH, K]),
                        op=OP.mult)
                    nc.vector.reduce_max(w3[:, :, j], mk3, axis=AX.X)
                # decode w -> plain previous-tag table F_s(x) = ptd[p, s*12+x]
                ptd = vpool.tile([128, LCH * K], FP, name="ptd")
                nc.vector.tensor_scalar(
                    out=ptd[:], in0=wptr[:], scalar1=-1.0, scalar2=float(K - 1),
                    op0=OP.mult, op1=OP.add)

                # --- 5e: backtrace map composition C_s = F_s o C_{s+1} ---
                Call = vpool.tile([128, LCH * K], FP, name="Call")
                nc.vector.tensor_copy(
                    Call[:, (LCH - 1) * K:], ptd[:, (LCH - 1) * K:])
                for s in range(LCH - 2, -1, -1):
                    oh = spool.tile([128, KK], FP, tag="co", name=f"co{s}")
                    oh3 = oh[:].rearrange("p (x y) -> p x y", y=K)
                    nc.vector.tensor_tensor(
                        out=oh3,
                        in0=Call[:, (s + 1) * K:(s + 2) * K].unsqueeze(2)
                            .broadcast_to([128, K, K]),
                        in1=io_sb[:].unsqueeze(1).broadcast_to([128, K, K]),
                        op=OP.is_equal)
                    nc.vector.tensor_tensor(
                        out=oh3, in0=oh3,
                        in1=ptd[:, s * K:(s + 1) * K].unsqueeze(1)
                            .broadcast_to([128, K, K]),
                        op=OP.mult)
                    nc.vector.reduce_max(
                        Call[:, s * K:(s + 1) * K], oh3, axis=AX.X)

                # --- 5f: init best tag (t = T-1) ---
                fs = spool.tile([BL, K], FP, tag="bt1", name="fs")
                nc.vector.tensor_add(fs[:], sfin[:], ts_sb[:])
                mx8 = spool.tile([BL, 8], FP, tag="mx8", name="mx8")
                nc.vector.max(mx8[:], fs[:])
                msk4 = spool.tile([BL, K], FP, tag="bmsk", name="msk4")
                nc.vector.tensor_scalar(
                    out=msk4[:], in0=fs[:], scalar1=mx8[:, 0:1], scalar2=None,
                    op0=OP.is_equal)
                nc.vector.tensor_mul(msk4[:], msk4[:], wv_sb[0:BL, :])
                w4 = spool.tile([BL, 1], FP, tag="w4", name="w4")
                nc.vector.reduce_max(w4[:], msk4[:], axis=AX.X)

                # --- 5g: serial chunk-boundary backtrace (x_c = end tag) ---
                nc.sync.dma_start(d_c0, Call[:, 0:K])
                c0b = vpool.tile([BL, CH * K], FP, name="c0b")
                nc.sync.dma_start(
                    c0b[:], d_c0.rearrange("(c b) j -> b c j", b=BL))
                xall = vpool.tile([BL, CH], FP, name="xall")
                nc.vector.tensor_scalar(
                    out=xall[:, CH - 1:CH], in0=w4[:], scalar1=-1.0,
                    scalar2=float(K - 1), op0=OP.mult, op1=OP.add)
                for c in range(CH - 1, 0, -1):
                    oh4 = spool.tile([BL, K], FP, tag="bo", name=f"bo{c}")
                    nc.vector.tensor_scalar(
                        out=oh4[:], in0=io_sb[0:BL, :],
                        scalar1=xall[:, c:c + 1], scalar2=None,
                        op0=OP.is_equal)
                    nc.vector.tensor_mul(
                        oh4[:], oh4[:], c0b[:, c * K:(c + 1) * K])
                    nc.vector.reduce_max(xall[:, c - 1:c], oh4[:], axis=AX.X)

                # --- 5h: batched tag application ---
                nc.sync.dma_start(d_x, xall[:])
                x128 = vpool.tile([128, 1], FP, name="x128")
                nc.sync.dma_start(x128[:], d_x.rearrange("b c -> c b"))
                oh128 = vpool.tile([128, K], FP, name="oh128")
                nc.vector.tensor_scalar(
                    out=oh128[:], in0=io_sb[:], scalar1=x128[:, 0:1],
                    scalar2=None, op0=OP.is_equal)
                tgt = spool.tile([128, LCH * K], FP, tag="tg", name="tgt")
                t3 = tgt[:].rearrange("p (s y) -> p s y", y=K)
                nc.vector.tensor_tensor(
                    out=t3,
                    in0=Call[:].rearrange("p (s y) -> p s y", y=K),
                    in1=oh128[:].unsqueeze(1).broadcast_to([128, LCH, K]),
                    op=OP.mult)
                tags = vpool.tile([128, LCH], FP, name="tags")
                nc.vector.reduce_max(tags[:], t3, axis=AX.X)
                pi_cb = vpool.tile([128, LCH], I32, name="pi_cb")
                nc.vector.tensor_copy(pi_cb[:], tags[:])
                xi = vpool.tile([BL, 1], I32, name="xi")
                nc.vector.tensor_copy(xi[:], xall[:, CH - 1:CH])

                # --- 5i: assemble path [b, t] ---
                # tags[p=(c,b), s] is the tag at t = 16c+s-1
                nc.sync.dma_start(
                    d_tags.rearrange("c b s -> (c b) s"), pi_cb[:])
                nc.sync.dma_start(
                    path_out[:, 0:LCH - 1], d_tags[0, :, 1:LCH])
                nc.sync.dma_start(
                    path_out[:, LCH - 1:T - 1],
                    d_tags.rearrange("c b s -> b c s")[:, 1:, :])
                nc.sync.dma_start(path_out[:, T - 1:T], xi[:])

    nc.compile()
    return nc


def prep_inputs(sentence, h0, c0, embed, W_ih_f, W_hh_f, b_f, W_ih_r, W_hh_r,
                b_r, W_out, b_out, transitions, T=512):
    """Host-side layout prep. Returns per-core input maps."""
    f32 = np.float32
    perm = np.r_[0:128, 128:256, 384:512, 256:384]  # i,f,g,o -> i,f,o,g
    gs = np.concatenate([np.full(128, s, f32) for s in (0.5, 0.5, 0.5, 1.0)])

    def prep_dir(W_ih, W_hh, b):
        Wi = np.asarray(W_ih, f32)[perm] * gs[:, None]
        bb = np.asarray(b, f32)[perm] * gs
        Wh = np.asarray(W_hh, f32)[perm] * (0.5 * gs)[:, None]
        return Wi.T.copy(), Wh.T.copy(), bb

    wihT_f, whhT_f, be_f = prep_dir(W_ih_f, W_hh_f, b_f)
    wihT_r, whhT_r, be_r = prep_dir(W_ih_r, W_hh_r, b_r)
    w_ihT = np.stack([wihT_f, wihT_r])
    w_hhT = np.stack([whhT_f, whhT_r]).astype(np.float16)
    b_in = np.stack([be_f.reshape(4, 128), be_r.reshape(4, 128)])  # [2,4,128]
    b_in = b_in.reshape(8, 128).T.copy()                           # [128,8]

    Wo = np.asarray(W_out, f32) * 0.5
    w_outT = np.stack([Wo[:, :128].T.copy(), Wo[:, 128:].T.copy()]).astype(np.float16)
    bout_rep = np.tile(np.asarray(b_out, f32)[None, :], (128, 1))

    tr = np.asarray(transitions, f32)
    trans128 = np.tile(tr.reshape(1, K * K), (128, 1))
    wvec128 = np.tile((K - 1 - np.arange(K, dtype=f32))[None, :], (128, 1))
    tstop = np.tile(tr[STOP][None, :], (BL, 1))
    s0 = np.full((BL, K), NEG, f32)
    s0[:, START] = 0.0
    ident = np.eye(128, dtype=f32)
    g0 = np.full((K, K), NEG, f32)
    np.fill_diagonal(g0, 0.0)
    g0_in = np.tile(g0.reshape(1, K * K), (128, 1))
    tr9_in = np.tile(tr[:, START][None, :], (BL, 1))
    iota_in = np.tile(np.arange(K, dtype=f32)[None, :], (128, 1))
    embed = np.asarray(embed, f32)
    sentence = np.asarray(sentence)

    maps = []
    for core in range(NCORES):
        sl = sentence[core * BL:(core + 1) * BL, :T].astype(np.int32)
        idx_tm = sl.T.reshape(-1)                       # n = t*BL+b
        idx_in = idx_tm.reshape(-1, 128).T.copy()       # [128, NTILE]
        h_i = 2.0 * np.asarray(h0, f32)[:, core * BL:(core + 1) * BL, :]
        c_i = 2.0 * np.asarray(c0, f32)[:, core * BL:(core + 1) * BL, :]
        maps.append({
            "idx_in": idx_in,
            "embed": embed,
            "w_ihT": w_ihT,
            "w_hhT": w_hhT,
            "b_in": b_in,
            "h_init": np.ascontiguousarray(h_i.transpose(0, 2, 1)).astype(np.float16),
            "c_init": np.ascontiguousarray(c_i.transpose(0, 2, 1)),
            "w_outT": w_outT,
            "bout_rep": bout_rep,
            "ident": ident,
            "identr": ident.astype(np.float16),
            "trans128": trans128,
            "wvec128": wvec128,
            "tstop": tstop,
            "scores0": s0,
            "g0_in": g0_in,
            "tr9_in": tr9_in,
            "iota_in": iota_in,
        })
    return maps


_NC_CACHE = {}


def kernel(sentence, h0, c0, embed, W_ih_f, W_hh_f, b_f, W_ih_r, W_hh_r, b_r,
           W_out, b_out, transitions):
    T = np.asarray(sentence).shape[1]
    if T not in _NC_CACHE:
        _NC_CACHE[T] = build_program(T)
    nc = _NC_CACHE[T]
    maps = prep_inputs(sentence, h0, c0, embed, W_ih_f, W_hh_f, b_f,
                       W_ih_r, W_hh_r, b_r, W_out, b_out, transitions, T=T)
    res = run_bass_kernel_spmd(nc, maps, list(range(NCORES)))
    out = np.concatenate([res.results[i]["path_out"] for i in range(NCORES)], axis=0)
    return out.astype(np.int32)
